# revision 3
# baseline (speedup 1.0000x reference)
"""Trainium2 Bass kernel for nn_AdditiveRecursiveNN (depth-13 binary tree of
64x64 matmuls with per-node weights gathered from a 50000x4096 table).

v2: fp8 DoubleRow design.

Sharding: data-parallel over the 16 depth-9 subtrees rooted at heap nodes
15..30 -- two subtrees per NeuronCore, concatenated level-major into one
"virtual tree" whose device levels hold 256/128/64/32 nodes (global tree
levels 11..8). The host packs W^T strips (x16, fp8e4m3), biases (x256, fp8),
and a pre-summed leaf stream s = relu(W_l)+relu(W_r) (x16, fp8); the device
runs every node as ONE DoubleRow fp8 matmul (0.5 cyc/col):

  psum[64, 64] = [W^T|W^T] (stride-0 dup) contracted with [h_l; h_r]
                 (adjacent slots of the previous level's h tile)

Level-11 nodes use the pre-summed s stream with a stride-0 rhs and
half-scaled weights (W*8 instead of W*16): W/2@s + W/2@s = W@s.
Biases are pre-accumulated into PSUM by DoubleRow K=1 ones-matmuls
(512 cols per 8-node group), so the drain is a single fused
relu(psum/16) on a rotating engine (scalar / vector / gpsimd).
DoubleRow outputs must start at partition 0, so everything is 64
partitions wide.

The top 8 global levels (255 nodes) run on the host in fp32; device fp8
error attenuates ~0.22x per host level, making the final loss error
negligible (~1e-6 relative).
"""
import sys
sys.path.insert(0, '/opt/trn_rl_repo')

import numpy as np
import ml_dtypes

E = 64
D = 13
NCORES = 8
LEVELS = [256, 128, 64, 32]        # virtual-tree nodes per device level
WT_SLOTS = sum(LEVELS)             # 480
S_SLOTS = 256                      # pre-summed leaf pairs (1:1 with level 0)
OUT_SLOTS = 32                     # last device level (global level 8)
SCALE = 16.0                       # h/s scale for fp8 range
F8 = ml_dtypes.float8_e4m3

_CACHE = {}


def _build_nc():
    import concourse.bacc as bacc
    import concourse.tile as tile
    import concourse.mybir as mybir

    f32 = mybir.dt.float32
    bf16 = mybir.dt.bfloat16
    fp8 = mybir.dt.float8e4
    DR = mybir.MatmulPerfMode.DoubleRow
    nc = bacc.Bacc(None, target_bir_lowering=False)

    wt = nc.dram_tensor("wt", [E, WT_SLOTS * E], fp8, kind="ExternalInput")
    sb = nc.dram_tensor("sb", [E, S_SLOTS * E], fp8, kind="ExternalInput")
    bi = nc.dram_tensor("bi", [1, WT_SLOTS * E], fp8, kind="ExternalInput")
    out = nc.dram_tensor("out", [E, OUT_SLOTS * E], bf16, kind="ExternalOutput")

    with tile.TileContext(nc) as tc:
        with (
            tc.tile_pool(name="str", bufs=1) as pool_s,
            tc.tile_pool(name="h", bufs=1) as pool_h,
            tc.tile_pool(name="ps", bufs=8, space="PSUM") as pool_ps,
        ):
            # constant lhsT for bias matmuls: [1, (t=2), 64] = ones | zeros
            ep = pool_s.tile([1, 2 * E], fp8, tag="ep")
            nc.vector.memset(ep[0:1, 0:E], 1.0)
            nc.vector.memset(ep[0:1, E:2 * E], 0.0)

            wt_t = pool_s.tile([E, WT_SLOTS * E], fp8, tag="wt")
            WCH = 60  # slots per DMA chunk (480/60 = 8 chunks)
            for t in range(0, WT_SLOTS, WCH):
                nc.sync.dma_start(wt_t[:, t * E:(t + WCH) * E],
                                  wt[:, t * E:(t + WCH) * E])
            sb_t = pool_s.tile([E, S_SLOTS * E], fp8, tag="sb")
            SCH = 64
            for t in range(0, S_SLOTS, SCH):
                nc.sync.dma_start(sb_t[:, t * E:(t + SCH) * E],
                                  sb[:, t * E:(t + SCH) * E])
            bi_t = pool_s.tile([1, WT_SLOTS * E], fp8, tag="bi")
            nc.sync.dma_start(bi_t[:, :], bi[:, :])

            ep_ap = ep[0:1, :].rearrange("p (t m) -> p t m", t=2)

            h_prev = None
            woff = 0      # slot offset into wt/bi streams
            drain_rot = 0
            out_t = pool_h.tile([E, OUT_SLOTS * E], bf16, tag="hout")
            for lvl, n in enumerate(LEVELS):
                last = lvl == len(LEVELS) - 1
                if not last:
                    h_new = pool_h.tile([E, n * E], fp8, tag=f"h{lvl}")
                for g0 in range(0, n, 8):
                    ps = pool_ps.tile([E, 8 * E], f32, tag="ps")
                    bsl = slice((woff + g0) * E, (woff + g0 + 8) * E)
                    nc.tensor.matmul(
                        out=ps[:, :], lhsT=ep_ap,
                        rhs=bi_t[0:1, bsl].unsqueeze(1).broadcast_to([1, 2, 8 * E]),
                        start=True, stop=False, perf_mode=DR,
                        skip_group_check=True)
                    for i in range(g0, g0 + 8):
                        lhsT = (wt_t[:, (woff + i) * E:(woff + i + 1) * E]
                                .unsqueeze(1).broadcast_to([E, 2, E]))
                        if lvl == 0:
                            rhs = (sb_t[:, i * E:(i + 1) * E]
                                   .unsqueeze(1).broadcast_to([E, 2, E]))
                        else:
                            rhs = (h_prev[:, 2 * i * E:(2 * i + 2) * E]
                                   .rearrange("p (t m) -> p t m", t=2))
                        nc.tensor.matmul(
                            out=ps[:, (i - g0) * E:(i - g0 + 1) * E],
                            lhsT=lhsT, rhs=rhs,
                            start=False, stop=True, perf_mode=DR,
                            skip_group_check=True)
                    dst = (out_t if last else h_new)[:, g0 * E:(g0 + 8) * E]
                    eng = drain_rot % 2
                    drain_rot += 1
                    if eng == 0:
                        nc.scalar.activation(
                            dst, ps[:, :],
                            func=mybir.ActivationFunctionType.Relu,
                            scale=1.0 / SCALE)
                    else:
                        nc.vector.tensor_scalar(
                            dst, ps[:, :], 1.0 / SCALE, 0.0,
                            mybir.AluOpType.mult, mybir.AluOpType.max)
                if not last:
                    h_prev = h_new
                woff += n
            nc.sync.dma_start(out[:, :], out_t[:, :])

    nc.compile()
    return nc


def _get_nc():
    if "nc" not in _CACHE:
        _CACHE["nc"] = _build_nc()
    return _CACHE["nc"]


def _pack_core(c, node_ids, emb, bias_table):
    """Pack wt/sb/bi streams for core c (half-trees rooted at heap nodes
    15+2c and 16+2c), virtual-tree level-major layout."""
    wt = np.empty((E, WT_SLOTS, E), dtype=np.float32)   # [k, slot, m]
    bi = np.empty((WT_SLOTS, E), dtype=np.float32)
    roots = (15 + 2 * c, 16 + 2 * c)
    woff = 0
    for lvl, n in enumerate(LEVELS):
        loc = 7 - lvl                 # local level in each half-tree
        nh = n // 2                   # nodes per half
        for q, g0 in enumerate(roots):
            start = (g0 + 1) * (1 << loc) - 1
            ids = node_ids[start:start + nh]
            block = emb[ids].reshape(nh, E, E)
            scale = SCALE * (0.5 if lvl == 0 else 1.0)
            sl = slice(woff + q * nh, woff + q * nh + nh)
            wt[:, sl, :] = block.transpose(2, 0, 1) * scale
            bi[sl, :] = bias_table[ids] * (SCALE * SCALE)
        woff += n
    # s stream: relu(leaf)+relu(leaf) pairs, 128 slots per half
    sb = np.empty((E, S_SLOTS, E), dtype=np.float32)    # [k, slot, n]
    for q, g0 in enumerate(roots):
        start = (g0 + 1) * 256 - 1
        ids = node_ids[start:start + 256]
        leaves = np.maximum(emb[ids].reshape(256, E, E), 0.0)
        s = (leaves[0::2] + leaves[1::2]) * SCALE       # [128, E, E]
        sb[:, q * 128:(q + 1) * 128, :] = s.transpose(1, 0, 2)
    return {
        "wt": np.ascontiguousarray(wt.reshape(E, WT_SLOTS * E)).astype(F8),
        "sb": np.ascontiguousarray(sb.reshape(E, S_SLOTS * E)).astype(F8),
        "bi": np.ascontiguousarray(bi.reshape(1, WT_SLOTS * E)).astype(F8),
    }


def _make_in_maps(np_inputs):
    node_ids = np.asarray(np_inputs["node_ids"]).astype(np.int64)
    emb = np.ascontiguousarray(np.asarray(np_inputs["embedding"], np.float32))
    bias_table = np.ascontiguousarray(
        np.asarray(np_inputs["bias_table"], np.float32))
    return [_pack_core(c, node_ids, emb, bias_table) for c in range(NCORES)]


def kernel(node_ids, label, embedding, bias_table, proj_w, proj_b):
    from concourse.bass_utils import run_bass_kernel_spmd

    node_ids = np.asarray(node_ids).astype(np.int64)
    emb = np.ascontiguousarray(np.asarray(embedding, dtype=np.float32))
    bias_table = np.ascontiguousarray(np.asarray(bias_table, dtype=np.float32))
    proj_w = np.asarray(proj_w, dtype=np.float32)
    proj_b = np.asarray(proj_b, dtype=np.float32)
    label_i = int(np.asarray(label))

    nc = _get_nc()
    in_maps = [_pack_core(c, node_ids, emb, bias_table) for c in range(NCORES)]
    res = run_bass_kernel_spmd(nc, in_maps, core_ids=list(range(NCORES)))

    # global level 8 h's from device outputs (out slots: [half A 16 | half B 16])
    h = np.empty((511, E, E), dtype=np.float32)  # heap nodes 0..510
    for c in range(NCORES):
        o = res.results[c]["out"].astype(np.float32) / SCALE  # [E, 32*E]
        o = o.reshape(E, OUT_SLOTS, E)
        for q, g0 in enumerate((15 + 2 * c, 16 + 2 * c)):
            base = (g0 + 1) * 16 - 1
            h[base:base + 16] = o[:, q * 16:(q + 1) * 16, :].transpose(1, 0, 2)

    # host: global levels 7..0 exact in fp32
    for lvl in range(7, -1, -1):
        start = (1 << lvl) - 1
        nn = 1 << lvl
        ids = node_ids[start:start + nn]
        W = emb[ids].reshape(nn, E, E)
        b = bias_table[ids]
        ch = h[2 * start + 1: 2 * start + 1 + 2 * nn]
        s = ch[0::2] + ch[1::2]
        h[start:start + nn] = np.maximum(W @ s + b[:, None, :], 0.0)

    root = h[0].reshape(-1)
    logits = root @ proj_w.T + proj_b
    m = logits.max()
    lse = m + np.log(np.exp(logits - m).sum())
    log_softmax = logits - lse
    loss = np.float32(-log_softmax[label_i])
    prediction = np.int64(np.argmax(logits))
    return prediction, loss


# revision 4
# speedup vs baseline: 1.7721x; 1.7721x over previous
"""v3: plain fp8 64x64 matmuls on alternating diagonal PE quadrants.

Per core: half-tree A lives on partitions 0:64 (PE quadrant (0,0)), half-tree
B on 64:128 (quadrant (64,64)); instructions alternate A/B so each quadrant's
LDWEIGHTS overlaps the other's MATMUL. Device levels hold 128/64/32/16 nodes
per half (global levels 11..8). Child sums for levels 10..8 are strided DVE
adds at full 128-partition width; level 11 consumes the host's pre-summed
relu-leaf stream. Bias is pre-accumulated into PSUM by one merged K=2
matmul per 8-slot group ([2,128] half-indicator lhsT). Drains are fused
relu(psum/16) on scalar/vector, full width. Host: packing (x16 fp8 scaling)
+ exact fp32 top-8 levels.
"""
import sys
sys.path.insert(0, '/opt/trn_rl_repo')

import numpy as np
import ml_dtypes

E = 64
NCORES = 8
HLEVELS = [128, 64, 32, 16]        # nodes per half-tree per device level
WT_SLOTS = sum(HLEVELS)            # 240 slots (per half; packed both halves)
S_SLOTS = 128                      # pre-summed leaf pairs per half
OUT_SLOTS = 16                     # last device level per half
SCALE = 16.0
F8 = ml_dtypes.float8_e4m3

_CACHE = {}


def _build_nc():
    import concourse.bacc as bacc
    import concourse.tile as tile
    import concourse.mybir as mybir

    f32 = mybir.dt.float32
    bf16 = mybir.dt.bfloat16
    fp8 = mybir.dt.float8e4
    nc = bacc.Bacc(None, target_bir_lowering=False)

    wt = nc.dram_tensor("wt", [128, WT_SLOTS * E], fp8, kind="ExternalInput")
    sb = nc.dram_tensor("sb", [128, S_SLOTS * E], fp8, kind="ExternalInput")
    bi = nc.dram_tensor("bi", [2, WT_SLOTS * E], bf16, kind="ExternalInput")
    ep = nc.dram_tensor("ep", [2, 128], bf16, kind="ExternalInput")
    out = nc.dram_tensor("out", [128, OUT_SLOTS * E], bf16,
                         kind="ExternalOutput")

    lo, hi = slice(0, E), slice(E, 128)

    with tile.TileContext(nc) as tc:
        with (
            tc.tile_pool(name="str", bufs=1) as pool_s,
            tc.tile_pool(name="h", bufs=1) as pool_h,
            tc.tile_pool(name="ps", bufs=8, space="PSUM") as pool_ps,
        ):
            ep_t = pool_s.tile([2, 128], bf16, tag="ep")
            nc.sync.dma_start(ep_t[:, :], ep[:, :])

            wt_t = pool_s.tile([128, WT_SLOTS * E], fp8, tag="wt")
            WCH = 60
            for t in range(0, WT_SLOTS, WCH):
                nc.sync.dma_start(wt_t[:, t * E:(t + WCH) * E],
                                  wt[:, t * E:(t + WCH) * E])
            sb_t = pool_s.tile([128, S_SLOTS * E], fp8, tag="sb")
            SCH = 64
            for t in range(0, S_SLOTS, SCH):
                nc.sync.dma_start(sb_t[:, t * E:(t + SCH) * E],
                                  sb[:, t * E:(t + SCH) * E])
            bi_t = pool_s.tile([2, WT_SLOTS * E], bf16, tag="bi")
            nc.sync.dma_start(bi_t[:, :], bi[:, :])

            h_prev = None
            woff = 0
            rot = 0
            out_t = pool_h.tile([128, OUT_SLOTS * E], bf16, tag="hout")
            for lvl, n in enumerate(HLEVELS):
                last = lvl == len(HLEVELS) - 1
                if not last:
                    h_new = pool_h.tile([128, n * E], fp8, tag=f"h{lvl}",
                                        name=f"h{lvl}")
                if lvl == 0:
                    s_cur = sb_t
                else:
                    # child sums for this level: n s-slots, strided DVE add
                    s_cur = pool_h.tile([128, n * E], fp8, tag=f"s{lvl}",
                                        name=f"s{lvl}")
                    for g0 in range(0, n, 8):
                        pairs = h_prev[:, 2 * g0 * E:2 * (g0 + 8) * E]
                        pv = pairs.rearrange("p (t c) -> p t c", c=2 * E)
                        nc.vector.tensor_add(
                            s_cur[:, g0 * E:(g0 + 8) * E].rearrange(
                                "p (t m) -> p t m", m=E),
                            pv[:, :, 0:E], pv[:, :, E:2 * E])
                for g0 in range(0, n, 8):
                    ps = pool_ps.tile([128, 8 * E], f32, tag="ps")
                    bsl = slice((woff + g0) * E, (woff + g0 + 8) * E)
                    nc.tensor.matmul(
                        out=ps[:, :], lhsT=ep_t[:, :], rhs=bi_t[:, bsl],
                        start=True, stop=False, skip_group_check=True)
                    for i in range(g0, g0 + 8):
                        wsl = slice((woff + i) * E, (woff + i + 1) * E)
                        ssl = slice(i * E, (i + 1) * E)
                        psl = slice((i - g0) * E, (i - g0 + 1) * E)
                        nc.tensor.matmul(
                            out=ps[lo, psl], lhsT=wt_t[lo, wsl],
                            rhs=s_cur[lo, ssl],
                            start=False, stop=True, tile_position=(0, 0),
                            skip_group_check=True)
                        nc.tensor.matmul(
                            out=ps[hi, psl], lhsT=wt_t[hi, wsl],
                            rhs=s_cur[hi, ssl],
                            start=False, stop=True, tile_position=(E, E),
                            skip_group_check=True)
                    dst = (out_t if last else h_new)[:, g0 * E:(g0 + 8) * E]
                    if rot % 3 != 2:   # scalar gets 2/3 (DVE also does adds)
                        nc.scalar.activation(
                            dst, ps[:, :],
                            func=mybir.ActivationFunctionType.Relu,
                            scale=1.0 / SCALE)
                    else:
                        nc.vector.tensor_scalar(
                            dst, ps[:, :], 1.0 / SCALE, 0.0,
                            mybir.AluOpType.mult, mybir.AluOpType.max)
                    rot += 1
                if not last:
                    h_prev = h_new
                woff += n
            nc.sync.dma_start(out[:, :], out_t[:, :])

    nc.compile()
    return nc


def _get_nc():
    if "nc" not in _CACHE:
        _CACHE["nc"] = _build_nc()
    return _CACHE["nc"]


def _pack_core(c, node_ids, emb, bias_table):
    """wt/sb/bi streams for core c; halves packed in partition dim."""
    wt = np.empty((2, E, WT_SLOTS, E), dtype=np.float32)   # [half, k, slot, m]
    bi = np.empty((2, WT_SLOTS, E), dtype=np.float32)
    sbuf = np.empty((2, E, S_SLOTS, E), dtype=np.float32)
    roots = (15 + 2 * c, 16 + 2 * c)
    for q, g0 in enumerate(roots):
        woff = 0
        for lvl, n in enumerate(HLEVELS):
            loc = 7 - lvl
            start = (g0 + 1) * (1 << loc) - 1
            ids = node_ids[start:start + n]
            block = emb[ids].reshape(n, E, E)
            wt[q, :, woff:woff + n, :] = block.transpose(2, 0, 1) * SCALE
            bi[q, woff:woff + n, :] = bias_table[ids] * (SCALE * SCALE)
            woff += n
        start = (g0 + 1) * 256 - 1
        ids = node_ids[start:start + 256]
        leaves = np.maximum(emb[ids].reshape(256, E, E), 0.0)
        s = (leaves[0::2] + leaves[1::2]) * SCALE
        sbuf[q] = s.transpose(1, 0, 2)
    return {
        "wt": np.ascontiguousarray(wt.reshape(128, WT_SLOTS * E)).astype(F8),
        "sb": np.ascontiguousarray(sbuf.reshape(128, S_SLOTS * E)).astype(F8),
        "bi": np.ascontiguousarray(
            bi.reshape(2, WT_SLOTS * E)).astype(ml_dtypes.bfloat16),
        "ep": _EP,
    }


_EP = np.zeros((2, 128), dtype=ml_dtypes.bfloat16)
_EP[0, 0:E] = 1.0
_EP[1, E:128] = 1.0


def _make_in_maps(np_inputs):
    node_ids = np.asarray(np_inputs["node_ids"]).astype(np.int64)
    emb = np.ascontiguousarray(np.asarray(np_inputs["embedding"], np.float32))
    bias_table = np.ascontiguousarray(
        np.asarray(np_inputs["bias_table"], np.float32))
    return [_pack_core(c, node_ids, emb, bias_table) for c in range(NCORES)]


def kernel(node_ids, label, embedding, bias_table, proj_w, proj_b):
    from concourse.bass_utils import run_bass_kernel_spmd

    node_ids = np.asarray(node_ids).astype(np.int64)
    emb = np.ascontiguousarray(np.asarray(embedding, dtype=np.float32))
    bias_table = np.ascontiguousarray(np.asarray(bias_table, dtype=np.float32))
    proj_w = np.asarray(proj_w, dtype=np.float32)
    proj_b = np.asarray(proj_b, dtype=np.float32)
    label_i = int(np.asarray(label))

    nc = _get_nc()
    in_maps = [_pack_core(c, node_ids, emb, bias_table) for c in range(NCORES)]
    res = run_bass_kernel_spmd(nc, in_maps, core_ids=list(range(NCORES)))

    h = np.empty((511, E, E), dtype=np.float32)
    for c in range(NCORES):
        o = res.results[c]["out"].astype(np.float32) / SCALE  # [128, 16*E]
        o = o.reshape(2, E, OUT_SLOTS, E)
        for q, g0 in enumerate((15 + 2 * c, 16 + 2 * c)):
            base = (g0 + 1) * 16 - 1
            h[base:base + 16] = o[q].transpose(1, 0, 2)

    for lvl in range(7, -1, -1):
        start = (1 << lvl) - 1
        nn = 1 << lvl
        ids = node_ids[start:start + nn]
        W = emb[ids].reshape(nn, E, E)
        b = bias_table[ids]
        ch = h[2 * start + 1: 2 * start + 1 + 2 * nn]
        s = ch[0::2] + ch[1::2]
        h[start:start + nn] = np.maximum(W @ s + b[:, None, :], 0.0)

    root = h[0].reshape(-1)
    logits = root @ proj_w.T + proj_b
    m = logits.max()
    lse = m + np.log(np.exp(logits - m).sum())
    log_softmax = logits - lse
    loss = np.float32(-log_softmax[label_i])
    prediction = np.int64(np.argmax(logits))
    return prediction, loss
